# revision 64
# baseline (speedup 1.0000x reference)
"""Multi-head cross-attention kernel for 8 Trainium2 NeuronCores (v2).

Sharding: core = (batch, head-group) — cores 0-3 take batch 0, cores 4-7
batch 1; core m%4 takes heads [4*(m%4), 4*(m%4)+4). Each core projects
q/k/v for its 4 heads, runs fused (no-max) softmax attention fully
on-chip, and produces a partial out-projection (transposed). The host
sums the four per-batch partials and transposes back.

v2 vs baseline (618us): all matmul operands bf16 (halves DMA + SBUF,
enables FWL fast weight load so LDWEIGHTS overlaps matmuls), K/V
projection of kv-groups 1-3 and the qh0 out-projection are interleaved
into the ACT-bound attention stream (exp on ScalarE is the 285us floor;
PE slack absorbs projection work), attnV PSUM accumulators are copied
to SBUF immediately at unit end (frees the 2-bank X ring fast), and
normalization (reciprocal ~8cyc/elem on DVE) runs off the PE critical
path with a dedicated 1-bank PSUM pool for the denominator broadcast.

Shapes (hardcoded per problem spec):
  query_states [2, 2048, 1024], key/value_states [2, 4096, 1024],
  Wq/Wk/Wv/Wo [1024, 1024] (torch Linear layout, applied as x @ W.T).

On-chip layouts per core (b = batch, hg = head group):
  xq = query[b].T [1024, 2048], xk/xv = key/value[b].T [1024, 4096]
  wqT/wkT/wvT = W[s,:].T -> [1024, 256], woT = Wo[:, s].T -> [256, 1024]
  QT per-head [128, 4, 2048] with pair-partner rows zeroed (keeps the
  score matmuls at full K=128 contraction; zero rows annihilate the
  partner head); KT pair-packed [128, 2, 4096]; Vsb [128, 32, 4, 66]
  memset to 1.0 so column 64 is the softmax-denominator ones column
  (V-proj copies fill columns 0:64; 65 is alignment pad).
  scores.T tiles [128 kv, 1024 q] -> exp via ACT (scale folded, no max
  subtraction: |scores/8| < 4 here) -> X[65, 512] += V'.T @ expS.
  Normalize: copy X to SBUF fp32, reciprocal of the whole tile (row 64
  is the denominator; other rows are harmless), partition-broadcast via
  a K=1 matmul against a ones row, multiply, then an SBUF->SBUF DMA
  moves the 64 rows to the head's partition base in Xsb (bf16).
  outT [1024, 2048] fp32 = woT.T @ Xsb, partial over the head slice.
"""

import numpy as np
import ml_dtypes

import concourse.tile as tile
from concourse import bacc, mybir
from concourse.bass_utils import run_bass_kernel_spmd
from concourse.tile_rust import add_dep_helper

B, QL, KVL, HIDDEN = 2, 2048, 4096, 1024
N_HEADS, HEAD_DIM = 16, 64
SCALE = HEAD_DIM**-0.5
N_CORES = 8
HPC = 4  # heads per core
DS = HPC * HEAD_DIM  # 256: per-core hidden slice

F32 = mybir.dt.float32
BF16 = mybir.dt.bfloat16

HC = HIDDEN // 128  # 8 contraction chunks over hidden
DC = DS // 128  # 2 chunks over the per-core 256-dim slice
KVC = KVL // 128  # 32 kv chunks
NGRP = 4  # kv groups of 1024 for the projection pipeline
GKV = KVL // NGRP  # 1024 kv per group


def _build_program():
    nc = bacc.Bacc(None)
    xq = nc.dram_tensor("xq", [HIDDEN, QL], BF16, kind="ExternalInput")
    xk = nc.dram_tensor("xk", [HIDDEN, KVL], BF16, kind="ExternalInput")
    xv = nc.dram_tensor("xv", [HIDDEN, KVL], BF16, kind="ExternalInput")
    # weights host-prearranged to the on-chip layout [128, HC*DS] /
    # [128, DC*HIDDEN] so the weight DMAs are contiguous (a strided
    # rearrange DMA costs ~4us of serial descriptor-issue time on the
    # sync queue; a contiguous one ~0.7us)
    wqP = nc.dram_tensor("wqP", [128, HC * DS], BF16, kind="ExternalInput")
    wkP = nc.dram_tensor("wkP", [128, HC * DS], BF16, kind="ExternalInput")
    wvP = nc.dram_tensor("wvP", [128, HC * DS], BF16, kind="ExternalInput")
    woP = nc.dram_tensor("woP", [128, DC * HIDDEN], BF16, kind="ExternalInput")
    outT = nc.dram_tensor("outT", [HIDDEN, QL], BF16, kind="ExternalOutput")

    with tile.TileContext(nc) as tc:
        with (
            tc.tile_pool(name="persist", bufs=1) as persist,
            tc.tile_pool(name="wpool", bufs=1) as wpool,
            # ring of 8 = exactly one group per tag: group g+1's DMAs are
            # gated by the ring dependency on group g's matmuls, keeping the
            # hardware DMA queues clear for the prefix-critical transfers
            tc.tile_pool(name="xkv", bufs=8) as xkv,
            tc.tile_pool(name="esb", bufs=3) as esb,
            tc.tile_pool(name="norm", bufs=3) as nsb,
            tc.tile_pool(name="outsb", bufs=2) as osb,
        ):
            # ---- persistent SBUF tensors ----
            # QT per-head with the pair-partner's 64 partitions zeroed: the
            # score matmuls contract the full K=128 (zero rows annihilate
            # the partner head). K=64 matmuls were measured 10% slower
            # end-to-end — the HAM activity monitor re-throttles the PE
            # clock when only half the array rows are active.
            KT = persist.tile([128, DC, KVL], BF16)
            QT = persist.tile([128, HPC, QL], BF16)
            Vsb = persist.tile([128, KVC, HPC, HEAD_DIM + 2], BF16)
            Xsb = persist.tile([128, DC, QL], BF16)
            ones_sb = persist.tile([128, HEAD_DIM], BF16)
            # ones FIRST: the PE's warm-up matmul reads it, and the DVE
            # queue serializes memsets — a big one ahead of it would
            # head-block the whole PE stream at startup
            nc.vector.memset(ones_sb, 1.0)
            nc.vector.memset(QT, 0.0)
            # only the denominator ones-column (and its pad) needs init
            nc.vector.memset(Vsb[:, :, :, HEAD_DIM : HEAD_DIM + 2], 1.0)

            # Weight DMAs are emitted in consumption order (wq before the Q
            # projection, wk/wv before group 0, wo last) so the first
            # matmuls aren't queued behind transfers they don't need.
            wq_sb = wpool.tile([128, HC, DS], BF16, tag="wq")
            wk_sb = wpool.tile([128, HC, DS], BF16, tag="wk")
            wv_sb = wpool.tile([128, HC, DS], BF16, tag="wv")
            wo_sb = wpool.tile([128, DC, HIDDEN], BF16, tag="wo")

            # xk/xv tiles, allocated in group order (12-deep ring per tag:
            # later groups reuse earlier slots once their matmuls have read
            # them). DMAs are emitted separately, in consumption order.
            xkt = {}
            xvt = {}
            for g in range(NGRP):
                for h in range(HC):
                    xkt[g, h] = xkv.tile(
                        [128, GKV], BF16, tag="xk", name=f"xk_{g}_{h}"
                    )
                    xvt[g, h] = xkv.tile(
                        [128, GKV], BF16, tag="xv", name=f"xv_{g}_{h}"
                    )

            # ---- prefix: Q projection + K/V group 0 (8 PSUM banks) ----
            with (
                tc.tile_pool(name="xqs", bufs=3) as xqs,
                tc.tile_pool(name="pproj8", bufs=8, space="PSUM") as p8,
            ):
                # pre-warm the exp table set during the prefix
                warm = nsb.tile([1, 2], BF16, tag="warm")
                wps = p8.tile([128, 512], F32, tag="p8", name="warm_ps")
                nc.tensor.matmul(
                    wps[0:1, 0:2], ones_sb[0:1, 0:1], ones_sb[0:1, 0:2],
                    start=True, stop=True,
                )
                nc.scalar.activation(
                    out=warm[:], in_=wps[0:1, 0:2],
                    func=mybir.ActivationFunctionType.Exp, scale=SCALE,
                )

                # Q projection, h-outer: 8 accumulators [128, 512]
                psq = [
                    [p8.tile([128, 512], F32, tag="p8", name=f"psq_{dq}_{t}") for t in range(4)]
                    for dq in range(DC)
                ]
                nc.sync.dma_start(wq_sb[:], wqP.rearrange("p (c m) -> p c m", c=HC))
                for h in range(HC):
                    xqt = xqs.tile([128, QL], BF16, tag="xq")
                    nc.sync.dma_start(xqt[:], xq[h * 128 : (h + 1) * 128, :])
                    for dq in range(DC):
                        for t in range(4):
                            nc.tensor.matmul(
                                psq[dq][t][:],
                                wq_sb[:, h, dq * 128 : (dq + 1) * 128],
                                xqt[:, t * 512 : (t + 1) * 512],
                                start=(h == 0),
                                stop=(h == HC - 1),
                            )
                for h in range(HPC):
                    pb = (h % 2) * 64
                    for t in range(4):
                        nc.vector.tensor_copy(
                            QT[pb : pb + 64, h, t * 512 : (t + 1) * 512],
                            psq[h // 2][t][pb : pb + 64, :],
                        )

                # K/V projection group 0, h-outer: 4+4 banks
                nc.sync.dma_start(wk_sb[:], wkP.rearrange("p (c m) -> p c m", c=HC))
                nc.sync.dma_start(wv_sb[:], wvP.rearrange("p (c m) -> p c m", c=HC))
                psk = [
                    [p8.tile([128, 512], F32, tag="p8", name=f"psk0_{dk}_{t}") for t in range(2)]
                    for dk in range(DC)
                ]
                psv = [
                    p8.tile([128, 512], F32, tag="p8", name=f"psv0_{c}")[:, :DS]
                    for c in range(8)
                ]
                last_pre_dma = None
                for h in range(HC):
                    xkt0 = xkt[0, h]
                    xvt0 = xvt[0, h]
                    nc.sync.dma_start(xkt0[:], xk[h * 128 : (h + 1) * 128, 0:GKV])
                    last_pre_dma = nc.sync.dma_start(
                        xvt0[:], xv[h * 128 : (h + 1) * 128, 0:GKV]
                    )
                    for dk in range(DC):
                        for t in range(2):
                            nc.tensor.matmul(
                                psk[dk][t][:],
                                wk_sb[:, h, dk * 128 : (dk + 1) * 128],
                                xkt0[:, t * 512 : (t + 1) * 512],
                                start=(h == 0),
                                stop=(h == HC - 1),
                            )
                    for c in range(8):
                        nc.tensor.matmul(
                            psv[c][:],
                            xvt0[:, c * 128 : (c + 1) * 128],
                            wv_sb[:, h, :],
                            start=(h == 0),
                            stop=(h == HC - 1),
                        )
                for dk in range(DC):
                    for t in range(2):
                        nc.vector.tensor_copy(
                            KT[:, dk, t * 512 : (t + 1) * 512], psk[dk][t][:]
                        )
                for c in range(8):
                    nc.vector.tensor_copy(
                        Vsb[:, c, :, 0:HEAD_DIM],
                        psv[c].rearrange("p (hh d) -> p hh d", hh=HPC),
                    )

                # remaining transfers, explicitly gated behind the last
                # prefix-critical transfer: the hardware DMA queues
                # round-robin, so without the dep these 12MB of prefetch
                # steal bandwidth from the data the first exp depends on
                nc.sync.dma_start(wo_sb[:], woP.rearrange("p (c m) -> p c m", c=DC))
                for g in range(1, NGRP):
                    for h in range(HC):
                        d1 = nc.sync.dma_start(
                            xkt[g, h][:],
                            xk[h * 128 : (h + 1) * 128, g * GKV : (g + 1) * GKV],
                        )
                        d2 = nc.sync.dma_start(
                            xvt[g, h][:],
                            xv[h * 128 : (h + 1) * 128, g * GKV : (g + 1) * GKV],
                        )
                        if g == 1 and last_pre_dma is not None:
                            for dd in (d1, d2):
                                if dd is not None:
                                    add_dep_helper(
                                        dd.ins,
                                        last_pre_dma.ins,
                                        reason="prefetch after prefix DMA",
                                    )

            # ---- attention units with interleaved projection/out-proj ----
            with (
                tc.tile_pool(name="pstg", bufs=2, space="PSUM") as pstg,
                tc.tile_pool(name="px", bufs=2, space="PSUM") as px,
                tc.tile_pool(name="pproj2", bufs=1, space="PSUM") as p2,
                tc.tile_pool(name="pbc", bufs=1, space="PSUM") as pbc,
            ):
                def k_tile_ops(g, dk, t):
                    """MMs + copy producing KT[:, dk, g*GKV + t*512 ...]."""
                    ops = []
                    state = {}

                    def mk_mm(h):
                        def op():
                            if h == 0:
                                state["ps"] = p2.tile(
                                    [128, 512], F32, tag="pk",
                                    name=f"psk{g}_{dk}_{t}",
                                )
                            nc.tensor.matmul(
                                state["ps"][:],
                                wk_sb[:, h, dk * 128 : (dk + 1) * 128],
                                xkt[g, h][:, t * 512 : (t + 1) * 512],
                                start=(h == 0),
                                stop=(h == HC - 1),
                            )
                        return op

                    for h in range(HC):
                        ops.append(mk_mm(h))

                    def cp():
                        nc.vector.tensor_copy(
                            KT[:, dk, g * GKV + t * 512 : g * GKV + (t + 1) * 512],
                            state["ps"][:],
                        )

                    ops.append(cp)
                    return ops

                def v_tile_ops(g, c):
                    """MMs + copy producing Vsb[:, g*8 + c, :, 0:64]."""
                    ops = []
                    state = {}

                    def mk_mm(h):
                        def op():
                            if h == 0:
                                state["ps"] = p2.tile(
                                    [128, 512], F32, tag="pk", name=f"psv{g}_{c}",
                                )
                            nc.tensor.matmul(
                                state["ps"][:, :DS],
                                xvt[g, h][:, c * 128 : (c + 1) * 128],
                                wv_sb[:, h, :],
                                start=(h == 0),
                                stop=(h == HC - 1),
                            )
                        return op

                    for h in range(HC):
                        ops.append(mk_mm(h))

                    def cp():
                        nc.vector.tensor_copy(
                            Vsb[:, g * 8 + c, :, 0:HEAD_DIM],
                            state["ps"][:, :DS].rearrange(
                                "p (hh d) -> p hh d", hh=HPC
                            ),
                        )

                    ops.append(cp)
                    return ops

                def proj_group_ops(g):
                    """Closures projecting K/V for group g. Tile dependencies
                    follow EMISSION order, so every op here must be emitted
                    before the first attention instruction that reads its
                    output — order by the absorbing unit's deadlines: that
                    unit consumes group g's kv chunks starting at its chunk
                    8*g, needing K dk0 t0 and all eight V chunks first; the
                    dk1 K tiles are only read by units h>=2, emitted later."""
                    ops = []
                    ops += k_tile_ops(g, 0, 0)
                    for c in range(8):
                        ops += v_tile_ops(g, c)
                    ops += k_tile_ops(g, 0, 1)
                    ops += k_tile_ops(g, 1, 0)
                    ops += k_tile_ops(g, 1, 1)
                    return ops

                def outproj_ops(tpair, pool=None, wide=False):
                    """Closures for the out-projection of q columns
                    [tpair*1024, (tpair+1)*1024) (needs Xsb for qh=tpair).
                    wide=True accumulates both 512-col halves in one 2-bank
                    PSUM tile and drains with a single copy — fewer DVE ops
                    for the latency-critical tail."""
                    ops = []
                    for oc in range(HIDDEN // 128):
                        state = {}

                        def mk_mm(dv, tt, oc=oc, state=state):
                            def op():
                                if wide:
                                    if dv == 0 and tt % 2 == 0:
                                        state["ps"] = pool.tile(
                                            [128, 1024], F32, tag="pkw",
                                            name=f"pso_{oc}",
                                        )
                                    dst = state["ps"][
                                        :, (tt % 2) * 512 : (tt % 2 + 1) * 512
                                    ]
                                else:
                                    if dv == 0:
                                        state[tt] = (pool or p2).tile(
                                            [128, 512], F32, tag="pk",
                                            name=f"pso_{oc}_{tt}",
                                        )
                                    dst = state[tt][:]
                                nc.tensor.matmul(
                                    dst,
                                    wo_sb[:, dv, oc * 128 : (oc + 1) * 128],
                                    Xsb[:, dv, tt * 512 : (tt + 1) * 512],
                                    start=(dv == 0),
                                    stop=(dv == DC - 1),
                                )
                            return op

                        def mk_cp(tt, oc=oc, state=state):
                            def op():
                                ot = state["ot"] = state.get("ot") or osb.tile(
                                    [128, 1024], BF16, tag="ot", name=f"ot_{oc}_{tpair}"
                                )
                                if wide:
                                    nc.vector.tensor_copy(ot[:], state["ps"][:])
                                else:
                                    nc.vector.tensor_copy(
                                        ot[:, (tt % 2) * 512 : (tt % 2 + 1) * 512],
                                        state[tt][:],
                                    )
                            return op

                        def mk_dma(oc=oc, state=state):
                            def op():
                                nc.sync.dma_start(
                                    outT[
                                        oc * 128 : (oc + 1) * 128,
                                        tpair * 1024 : (tpair + 1) * 1024,
                                    ],
                                    state["ot"][:],
                                )
                            return op

                        for tt in (tpair * 2, tpair * 2 + 1):
                            for dv in range(DC):
                                ops.append(mk_mm(dv, tt))
                            if not wide or tt % 2 == 1:
                                ops.append(mk_cp(tt))
                        ops.append(mk_dma())
                    return ops

                units = [(qh, h) for qh in range(2) for h in range(HPC)]
                # SBUF fp32 accumulators for the attnV output: kv groups are
                # the OUTER loop (so group g+1's projection is always fully
                # emitted during sweep g — Tile dependencies follow emission
                # order), which means all 8 units accumulate simultaneously;
                # PSUM can only hold one unit's accumulator, so each group's
                # partial is folded into SBUF here.
                Xacc = [
                    persist.tile([65, QL // 2], F32, name=f"xacc_{ui}")
                    for ui in range(len(units))
                ]

                def finalize_unit_ops(ui, qh, h, rr, row, pool=None):
                    """Broadcast 1/denom (row `row` of recip tile rr) across
                    64 partitions via a K=1 matmul, scale the numerators,
                    DMA into Xsb. Returned as closures so the PE-visible ops
                    can be pumped into later units' chunk streams instead of
                    head-blocking the engine queues between units."""
                    hc, pb = h // 2, (h % 2) * 64
                    q0 = qh * 1024
                    ops = []
                    for t in range(2):
                        state = {}

                        def mk_bc(t=t, state=state):
                            def op():
                                bc = state["bc"] = (pool or pbc).tile(
                                    [64, 512], F32, tag="bc", name=f"bc_{ui}_{t}"
                                )
                                # tile_position passed explicitly: bass's
                                # auto-derive rejects base partition 96
                                nc.tensor.matmul(
                                    bc[:],
                                    ones_sb[row : row + 1, 0:64],
                                    rr[row : row + 1, t * 512 : (t + 1) * 512],
                                    start=True,
                                    stop=True,
                                    tile_position=(row, 0),
                                )
                            return op

                        def mk_scale(t=t, state=state):
                            def op():
                                sc = nsb.tile(
                                    [64, 512], BF16, tag="sc", name=f"sc_{ui}_{t}"
                                )
                                nc.vector.tensor_tensor(
                                    sc[:],
                                    Xacc[ui][0:64, t * 512 : (t + 1) * 512],
                                    state["bc"][:],
                                    mybir.AluOpType.mult,
                                )
                                nc.sync.dma_start(
                                    Xsb[
                                        pb : pb + 64,
                                        hc,
                                        q0 + t * 512 : q0 + (t + 1) * 512,
                                    ],
                                    sc[:],
                                )
                            return op

                        ops.append(mk_bc())
                        ops.append(mk_scale())
                    return ops

                def batch_recip(rp, rr):
                    # rows {0,32,64,96} hold packed denominators; the other
                    # rows' reciprocals are computed and ignored (DVE recip
                    # cost is free-dim-bound, extra partitions are free)
                    with nc.allow_low_precision(
                        reason="1/denom as bf16 is a 0.2% scale wobble, "
                        "well inside the 2e-2 tolerance"
                    ):
                        nc.vector.reciprocal(rr[:], rp[:])

                # denominator pack tiles: rows {0,32,64,96} (the partition
                # bases a K=1 broadcast matmul can read from) hold units
                # 0-3 / 4-6 / 7's denominator rows, so one wide reciprocal
                # serves several units.
                rpA = persist.tile([97, 1024], F32, name="rpA")
                rrA = persist.tile([97, 1024], BF16, name="rrA")
                rpB = persist.tile([65, 1024], F32, name="rpB")
                rrB = persist.tile([65, 1024], BF16, name="rrB")
                rr7 = persist.tile([65, 1024], BF16, name="rr7")

                for g in range(NGRP):
                    # work pumped into this sweep's chunk iterations: K/V
                    # projection of the NEXT group (sweeps 0-2), then the
                    # normalization + qh0 out-projection (sweep 3).
                    if g < NGRP - 1:
                        filler, rate, delay = proj_group_ops(g + 1), 3, 0
                    else:
                        filler, rate, delay = [], 6, 0
                    for ui, (qh, h) in enumerate(units):
                        hc = h // 2
                        q0 = qh * 1024
                        X = [
                            px.tile([65, 512], F32, tag="x", name=f"X_{ui}_{g}_{t}")
                            for t in range(2)
                        ]
                        for c in range(g * 8, (g + 1) * 8):
                            stg = pstg.tile(
                                [128, 1024], F32, tag="stg", name=f"stg_{ui}_{c}"
                            )
                            for t in range(2):
                                nc.tensor.matmul(
                                    stg[:, t * 512 : (t + 1) * 512],
                                    KT[:, hc, c * 128 : (c + 1) * 128],
                                    QT[:, h, q0 + t * 512 : q0 + (t + 1) * 512],
                                    start=True,
                                    stop=True,
                                )
                            eS = esb.tile(
                                [128, 1024], BF16, tag="es", name=f"es_{ui}_{c}"
                            )
                            nc.scalar.activation(
                                out=eS[:],
                                in_=stg[:],
                                func=mybir.ActivationFunctionType.Exp,
                                scale=SCALE,
                            )
                            for t in range(2):
                                nc.tensor.matmul(
                                    X[t][:],
                                    Vsb[:, c, h, 0 : HEAD_DIM + 1],
                                    eS[:, t * 512 : (t + 1) * 512],
                                    start=(c == g * 8),
                                    stop=(c == (g + 1) * 8 - 1),
                                )
                            if filler and delay == 0:
                                for _ in range(rate):
                                    if filler:
                                        filler.pop(0)()
                            elif delay:
                                delay -= 1
                        # fold this group's partial into the SBUF accumulator
                        for t in range(2):
                            tsl = slice(t * 512, (t + 1) * 512)
                            if g == 0:
                                nc.vector.tensor_copy(Xacc[ui][:, tsl], X[t][:])
                            else:
                                nc.vector.tensor_tensor(
                                    Xacc[ui][:, tsl],
                                    X[t][:],
                                    Xacc[ui][:, tsl],
                                    mybir.AluOpType.add,
                                )
                        if g == NGRP - 1:
                            # pack this unit's denominator row for a batched
                            # reciprocal; finalize + out-proj are pumped into
                            # the remaining units' chunk streams.
                            if ui < 4:
                                nc.sync.dma_start(
                                    rpA[32 * ui : 32 * ui + 1, :],
                                    Xacc[ui][64:65, :],
                                )
                            elif ui < 7:
                                nc.sync.dma_start(
                                    rpB[32 * (ui - 4) : 32 * (ui - 4) + 1, :],
                                    Xacc[ui][64:65, :],
                                )
                            if ui == 3:
                                batch_recip(rpA, rrA)
                                for uj in range(4):
                                    filler += finalize_unit_ops(
                                        uj, *units[uj], rrA, 32 * uj
                                    )
                                filler += outproj_ops(0)
                                delay = 5  # let the recip clear the DVE
                                           # before the first bc matmul
                            if ui == 6:
                                batch_recip(rpB, rrB)
                                for uj in range(4, 7):
                                    filler += finalize_unit_ops(
                                        uj, *units[uj], rrB, 32 * (uj - 4)
                                    )
                                delay = 5  # recipB must clear the DVE first
                            # HAM bridge: the unit boundary stalls ~2-7us on
                            # the DVE (fold + pack + batched recip); a few
                            # dependency-free matmuls keep the PE activity
                            # monitor from re-throttling the clock to 1.2GHz
                            for i in range(6):
                                dmy = pbc.tile(
                                    [64, 512], F32, tag="bc",
                                    name=f"dmy3_{ui}_{i}",
                                )
                                nc.tensor.matmul(
                                    dmy[:], ones_sb[0:1, 0:64], QT[0:1, 0, 0:512],
                                    start=True, stop=True,
                                )
                    while filler:
                        filler.pop(0)()

            # tail: unit 7 solo (recip straight off its accumulator — rows
            # 0:64 are ignored), then out-proj of q [1024, 2048). A fresh
            # PSUM pool (the sweep pools just closed, all 8 banks free)
            # gives the out-proj chains a 4-deep ring so the matmuls
            # pipeline instead of serializing on each PSUM->SBUF copy.
            with (
                tc.tile_pool(name="ptailw", bufs=3, space="PSUM") as ptailw,
                tc.tile_pool(name="ptail", bufs=2, space="PSUM") as ptail,
            ):
                # dummy matmuls with no dependencies: they execute right
                # after the last attnV, bridging the ~7us DVE normalize
                # latency so HAM doesn't re-throttle the PE for the final
                # out-projection matmuls.
                for i in range(16):
                    dmy = ptail.tile([64, 512], F32, tag="bc", name=f"dmy_{i}")
                    nc.tensor.matmul(
                        dmy[:], ones_sb[0:1, 0:64], QT[0:1, 0, 0:512],
                        start=True, stop=True,
                    )
                # unit 7 solo: recip per q-half straight off its accumulator
                # so the first broadcast fires after ~3.3us, not 6.5
                fin7 = finalize_unit_ops(7, *units[7], rr7, 64, pool=ptail)
                for t in range(2):
                    with nc.allow_low_precision(reason="bf16 1/denom"):
                        nc.vector.reciprocal(
                            rr7[64:65, t * 512 : (t + 1) * 512],
                            Xacc[7][64:65, t * 512 : (t + 1) * 512],
                        )
                    fin7.pop(0)()  # broadcast
                    fin7.pop(0)()  # scale + move
                for op in outproj_ops(1, pool=ptailw, wide=True):
                    op()

    nc.finalize()
    return nc


_PROGRAM = None


def _program():
    global _PROGRAM
    if _PROGRAM is None:
        _PROGRAM = _build_program()
    return _PROGRAM


def _shard_inputs(query_states, key_states, value_states, Wq, Wk, Wv, Wo):
    bf = ml_dtypes.bfloat16
    xqs = [np.ascontiguousarray(query_states[b].T).astype(bf) for b in range(B)]
    xks = [np.ascontiguousarray(key_states[b].T).astype(bf) for b in range(B)]
    xvs = [np.ascontiguousarray(value_states[b].T).astype(bf) for b in range(B)]
    def wpack(wT, chunks):
        # [chunks*128, m] -> [128, chunks*m]: on-chip layout, so the weight
        # DMA is a single contiguous transfer
        m = wT.shape[1]
        return np.ascontiguousarray(
            wT.reshape(chunks, 128, m).transpose(1, 0, 2).reshape(128, chunks * m)
        ).astype(bf)

    in_maps = []
    for core in range(N_CORES):
        b = core // HPC
        hg = core % HPC
        s = slice(hg * DS, (hg + 1) * DS)
        in_maps.append(
            {
                "xq": xqs[b],
                "xk": xks[b],
                "xv": xvs[b],
                "wqP": wpack(Wq[s, :].T, HC),
                "wkP": wpack(Wk[s, :].T, HC),
                "wvP": wpack(Wv[s, :].T, HC),
                "woP": wpack(Wo[:, s].T, DC),
            }
        )
    return in_maps


def _gather_output(results):
    out = np.empty((B, QL, HIDDEN), np.float32)
    for b in range(B):
        acc = results[b * HPC]["outT"].astype(np.float32)
        for i in range(1, HPC):
            acc = acc + results[b * HPC + i]["outT"]
        out[b] = acc.T
    return out


def run_sharded(inputs, trace=False, tmpdir=None):
    """Run the SPMD kernel; returns (full_output, BassKernelResults)."""
    arrs = {k: np.asarray(v, dtype=np.float32) for k, v in inputs.items()}
    in_maps = _shard_inputs(
        arrs["query_states"],
        arrs["key_states"],
        arrs["value_states"],
        arrs["Wq"],
        arrs["Wk"],
        arrs["Wv"],
        arrs["Wo"],
    )
    res = run_bass_kernel_spmd(
        _program(), in_maps, list(range(N_CORES)), trace=trace, tmpdir=tmpdir
    )
    return _gather_output(res.results), res


def kernel(**inputs):
    out, _ = run_sharded(inputs)
    return out
